# revision 24
# baseline (speedup 1.0000x reference)
"""Trainium2 Bass kernel for nn_Attention_16355235463288.

Additive attention:
    q  = ht_query @ Wq.T                      [B, D]
    e  = tanh(ctx_key + q[:, None, None, :])  [B, H, W, D]
    s  = einsum('bhwd,d->bhw', e, Wa[0]) + ba [B, H, W]
    s  = exp(s - max(s)) * mask ; s /= (sum_hw(s) + 1e-10)
    ct = einsum('bchw,bhw->bc', ctx_val, s)   [B, C]
    returns (ct, s)

Sharding: pure data parallel, B=32 over 8 cores (BL=4 per core). Params
replicated. No collectives. The global max-subtract in the reference is
dropped: softmax ratios are invariant to the subtracted constant except
through the +1e-10 term, where the relative effect is ~1e-11; |s| is
bounded by sum|Wa| ~ 20 so exp() cannot overflow in fp32. The mask is
folded in as a host-precomputed log(mask) initializer of the score
accumulator, so exp(s + logmask) = exp(s) * mask with no mask multiply.

Per-core dataflow (each stage streams ~33.5 MB from HBM), per batch b so
stage 2 of batch b overlaps stage 1 of batch b+1:
  stage 0: qT[d, b] via PE matmul from host-pretransposed WqT/htT (fp32).
  stage 1 (ctx_key):
    SWDGE cast-loads key tiles [128(hw), 512(d)] f32->bf16
    -> PE transpose 128x128 bf16 blocks into PSUM [128(d), hw]
    -> ACT tanh(psum + qT_bias) -> bf16 SBUF (fused q-add)
    -> PE matmul with zero-padded-column Wa weights -> scores [4, 512]
    -> DVE in-place add into s_sb[4, HW] (rows != b get += 0).
  softmax(b): ACT exp(s_sb + ba) -> u bf16 with accum_out giving the
    denominator for free; DVE reciprocal; tiny PE matmul broadcasts
    recip[b] over 128 partitions.
  stage 2 (ctx_val):
    PE selector matmul broadcasts u[b] over 128 partitions
    -> ACT Copy with scale=recip128 (fused normalize) -> ubc bf16
    -> SWDGE cast-load val tiles [128(c), 4096(hw)] f32->bf16
    -> DVE mul + reduce over hw -> ct column.
  tail: one fp32 exp + normalize for the s output.

tensor_tensor_reduce is avoided: it faults the NRT runtime on this
fleet (bisected 2026-08-03); plain tensor_tensor + tensor_reduce works.
"""

import sys
import numpy as np

for _p in ("/opt/trn_rl_repo", "/opt/pypackages"):
    if _p not in sys.path:
        sys.path.append(_p)

B, H, W, D, N, C = 32, 64, 64, 512, 512, 512
NCORES = 8
BL = B // NCORES          # 4 batches per core
HW = H * W                # 4096
G = 1024                  # hw positions per stage-1 group
NG = HW // G              # 4 groups
TPG = G // 128            # 8 hw-tiles per group

_CACHE = {}


def _build_nc():
    import concourse.bass as bass
    import concourse.mybir as mybir
    import concourse.tile as tile
    from concourse import bacc
    from concourse.masks import make_identity
    from contextlib import ExitStack

    f32 = mybir.dt.float32
    bf16 = mybir.dt.bfloat16
    AF = mybir.ActivationFunctionType
    OP = mybir.AluOpType

    nc = bacc.Bacc(None, target_bir_lowering=False, debug=False)

    key_d = nc.declare_dram_parameter("key", [BL, HW, D], f32, isOutput=False)
    val_d = nc.declare_dram_parameter("val", [BL, C, HW], f32, isOutput=False)
    lmask_d = nc.declare_dram_parameter("lmask", [BL, HW], f32, isOutput=False)
    htT_d = nc.declare_dram_parameter("htT4", [128, 4, BL], f32, isOutput=False)
    wqT_d = nc.declare_dram_parameter("WqT4", [128, 4, D], f32, isOutput=False)
    wap_d = nc.declare_dram_parameter("wa_pad", [128, BL, 4, BL], f32, isOutput=False)
    sel_d = nc.declare_dram_parameter("sel", [BL, BL, 128], f32, isOutput=False)
    ba_d = nc.declare_dram_parameter("ba4", [BL, 1], f32, isOutput=False)
    dg_d = nc.declare_dram_parameter("diag4", [BL, BL * 4], f32, isOutput=False)
    ct_d = nc.declare_dram_parameter("out_ct", [BL, C], f32, isOutput=True)
    s_d = nc.declare_dram_parameter("out_s", [BL, HW], f32, isOutput=True)

    with tile.TileContext(nc) as tc, ExitStack() as ctx:
        const = ctx.enter_context(tc.tile_pool(name="const", bufs=1))
        keyp = ctx.enter_context(tc.tile_pool(name="keyp", bufs=4))
        eTp = ctx.enter_context(tc.tile_pool(name="eTp", bufs=2))
        valp = ctx.enter_context(tc.tile_pool(name="valp", bufs=10))
        sbcp = ctx.enter_context(tc.tile_pool(name="sbcp", bufs=2))
        scrp = ctx.enter_context(tc.tile_pool(name="scrp", bufs=2))
        smallp = ctx.enter_context(tc.tile_pool(name="smallp", bufs=1))
        up = ctx.enter_context(tc.tile_pool(name="up", bufs=2))
        psT = ctx.enter_context(tc.tile_pool(name="psT", bufs=2, space="PSUM"))
        psS = ctx.enter_context(tc.tile_pool(name="psS", bufs=2, space="PSUM"))
        psB = ctx.enter_context(tc.tile_pool(name="psB", bufs=2, space="PSUM"))

        identity = const.tile([128, 128], bf16)
        make_identity(nc, identity)
        htT = const.tile([128, 4, BL], f32)
        nc.sync.dma_start(htT[:], htT_d[:])
        wqT = const.tile([128, 4, D], f32)
        nc.sync.dma_start(wqT[:], wqT_d[:])
        wap = const.tile([128, BL, 4, BL], bf16)
        nc.gpsimd.dma_start(wap[:], wap_d[:])
        sel = const.tile([BL, BL, 128], bf16)
        nc.gpsimd.dma_start(sel[:], sel_d[:])
        ba4 = const.tile([BL, 1], f32)
        nc.sync.dma_start(ba4[:], ba_d[:])
        diag4 = const.tile([BL, BL * 4], f32)
        nc.sync.dma_start(diag4[:], dg_d[:])
        ones4 = const.tile([BL, 128], f32)
        nc.vector.memset(ones4[:], 1.0)
        ct_sb = const.tile([128, BL * 4], f32)
        ctp = const.tile([128, BL * 4, 2], f32)

        # stage 0: qT[d_local, (k, b)] = sum_n Wq[d, n] * ht[b, n]
        ps_q = psS.tile([128, 4 * BL], f32, tag="pss")
        for k in range(4):
            for nk in range(4):
                nc.tensor.matmul(
                    ps_q[:, k * BL:(k + 1) * BL],
                    wqT[:, nk, k * 128:(k + 1) * 128],
                    htT[:, nk, :],
                    start=(nk == 0),
                    stop=(nk == 3),
                )
        qT = const.tile([128, 4 * BL], f32)
        nc.vector.tensor_copy(qT[:], ps_q[:])

        # score accumulator, initialized with log(mask) so the mask is
        # applied for free by the exp
        s_sb = smallp.tile([BL, HW], f32, tag="s_sb")
        nc.sync.dma_start(s_sb[:], lmask_d[:])
        denom = smallp.tile([BL, 1], f32, tag="denom")
        recip = smallp.tile([BL, 1], f32, tag="recip")

        HH = HW // 2  # 2048, half of the hw axis
        for b in range(BL):
            val_tiles = {}
            for half in range(2):
                # ---- stage 1 for this half (2 groups of G=1024) ----
                for gi in range(2):
                    g = half * 2 + gi
                    key_sb = keyp.tile([128, TPG, D], bf16, tag="key")
                    nc.gpsimd.dma_start(
                        key_sb[:],
                        key_d[b, g * G:(g + 1) * G, :].rearrange(
                            "(t p) d -> p t d", p=128
                        ),
                    )
                    # prefetch val halves: 2 per group -> 8 per batch
                    for vi in range(2):
                        cc = gi * 2 + vi
                        vt = valp.tile(
                            [128, HH], bf16, tag="val",
                            name=f"val_{b}_{half}_{cc}",
                        )
                        nc.gpsimd.dma_start(
                            vt[:],
                            val_d[b, cc * 128:(cc + 1) * 128,
                                  half * HH:(half + 1) * HH],
                        )
                        val_tiles[(half, cc)] = vt
                    eT = eTp.tile([128, 4, G], bf16, tag="eT")
                    for k in range(4):
                        psT_t = psT.tile([128, G], bf16, tag="pst")
                        for t in range(TPG):
                            nc.tensor.transpose(
                                psT_t[:, t * 128:(t + 1) * 128],
                                key_sb[:, t, k * 128:(k + 1) * 128],
                                identity,
                            )
                        nc.scalar.activation(
                            eT[:, k, :],
                            psT_t[:],
                            AF.Tanh,
                            bias=qT[:, (k * BL + b):(k * BL + b) + 1],
                        )
                    for c in range(G // 512):
                        ps_s = psS.tile([BL, 512], f32, tag="pss")
                        for k in range(4):
                            nc.tensor.matmul(
                                ps_s[:],
                                wap[:, b, k, :],
                                eT[:, k, c * 512:(c + 1) * 512],
                                start=(k == 0),
                                stop=(k == 3),
                            )
                        off = g * G + c * 512
                        # rows != b of ps_s are zero (zero-padded Wa cols),
                        # so an in-place add deposits row b w/o clobbering
                        nc.vector.tensor_tensor(
                            s_sb[:, off:off + 512],
                            s_sb[:, off:off + 512],
                            ps_s[:],
                            OP.add,
                        )

                # ---- unnormalized softmax numerator for this half ----
                # (rows != b hold junk masked by the selector matmul)
                u = up.tile([BL, HH], bf16, tag="u")
                nc.scalar.activation(
                    u[:], s_sb[:, half * HH:(half + 1) * HH],
                    AF.Exp, bias=ba4[:],
                )
                sbc = sbcp.tile([128, HH], bf16, tag="sbc")
                for h2 in range(2):
                    ps_b = psB.tile([128, 1024], f32, tag="psb")
                    for j in range(2):
                        off = h2 * 1024 + j * 512
                        nc.tensor.matmul(
                            ps_b[:, j * 512:(j + 1) * 512],
                            sel[:, b, :],
                            u[:, off:off + 512],
                            start=True,
                            stop=True,
                        )
                    nc.scalar.activation(
                        sbc[:, h2 * 1024:(h2 + 1) * 1024], ps_b[:], AF.Copy
                    )

                # ---- stage 2 partial reductions for this half ----
                for cc in range(4):
                    val_sb = val_tiles[(half, cc)]
                    prod = scrp.tile([128, HH], bf16, tag="prod")
                    sum2 = scrp.tile([128, HH // 2], bf16, tag="sum2")
                    idx = b * 4 + cc
                    nc.vector.tensor_tensor(
                        prod[:], val_sb[:], sbc[:], OP.mult
                    )
                    nc.vector.tensor_tensor(
                        sum2[:], prod[:, 0:HH // 2], prod[:, HH // 2:HH],
                        OP.add,
                    )
                    nc.vector.tensor_reduce(
                        ctp[:, idx, half:half + 1],
                        sum2[:],
                        mybir.AxisListType.X,
                        OP.add,
                    )

        # ---- tail: fp32 normalized s output + denominators ----
        s_e = smallp.tile([BL, HW], f32, tag="s_e")
        nc.scalar.activation(
            s_e[:], s_sb[:], AF.Exp, bias=ba4[:], accum_out=denom[:]
        )
        nc.vector.reciprocal(recip[:], denom[:])
        nc.vector.tensor_scalar_mul(s_sb[:], s_e[:], recip[:])
        nc.sync.dma_start(s_d[:], s_sb[:])
        nc.vector.tensor_tensor(
            ct_sb[:], ctp[:, :, 0], ctp[:, :, 1], OP.add
        )

        # normalize ct: recip_cols[p, (b,cc)] = recip[b]
        rdiag = smallp.tile([BL, BL * 4], f32, tag="rdiag")
        nc.vector.tensor_scalar_mul(rdiag[:], diag4[:], recip[:])
        ps_n = psB.tile([128, 1024], f32, tag="psb")
        nc.tensor.matmul(
            ps_n[:, 0:BL * 4], ones4[:], rdiag[:], start=True, stop=True
        )
        nc.vector.tensor_tensor(
            ct_sb[:], ct_sb[:], ps_n[:, 0:BL * 4], OP.mult
        )
        nc.sync.dma_start(
            ct_d.rearrange("b (c p) -> p b c", p=128),
            ct_sb.rearrange("p (b c) -> p b c", b=BL),
        )

    nc.compile()
    return nc


def _prep_in_maps(ctx_val, ctx_key, ctx_mask, ht_query, Wq, Wa, ba):
    key = np.ascontiguousarray(ctx_key.reshape(B, HW, D), dtype=np.float32)
    val = np.ascontiguousarray(ctx_val.reshape(B, C, HW), dtype=np.float32)
    mask = np.asarray(ctx_mask, dtype=np.float32).reshape(B, HW)
    with np.errstate(divide="ignore"):
        lmask = np.where(mask > 0, np.log(np.maximum(mask, 1e-38)), -1e30)
    lmask = np.ascontiguousarray(lmask, dtype=np.float32)
    ht = np.asarray(ht_query, dtype=np.float32)
    Wq = np.asarray(Wq, dtype=np.float32)
    Wa = np.asarray(Wa, dtype=np.float32)
    ba = np.asarray(ba, dtype=np.float32)

    # WqT4[n_local, nk, d] = Wq[d, nk*128 + n_local]
    WqT4 = np.ascontiguousarray(
        Wq.T.reshape(4, 128, D).transpose(1, 0, 2), dtype=np.float32
    )
    # wa_pad[d_local, b, k, col] = Wa[0, k*128+d_local] if col == b else 0
    waT = Wa[0].reshape(4, 128).T  # [d_local, k]
    wa_pad = np.zeros((128, BL, 4, BL), dtype=np.float32)
    for b in range(BL):
        wa_pad[:, b, :, b] = waT
    # sel[b, p, m] = 1 if p == b
    sel = np.zeros((BL, BL, 128), dtype=np.float32)
    for b in range(BL):
        sel[b, b, :] = 1.0
    ba4 = np.full((BL, 1), float(ba[0]), dtype=np.float32)
    # diag4[p, (b, cc)] = 1 if p == b (for the end-of-kernel ct normalize)
    diag4 = np.zeros((BL, BL * 4), dtype=np.float32)
    for b in range(BL):
        diag4[b, b * 4:(b + 1) * 4] = 1.0

    in_maps = []
    for core in range(NCORES):
        sl = slice(core * BL, (core + 1) * BL)
        ht_sh = ht[sl]  # [BL, N]
        # htT4[n_local, nk, b] = ht_sh[b, nk*128 + n_local]
        htT4 = np.ascontiguousarray(
            ht_sh.T.reshape(4, 128, BL).transpose(1, 0, 2), dtype=np.float32
        )
        in_maps.append(
            {
                "key": key[sl],
                "val": val[sl],
                "lmask": lmask[sl],
                "htT4": htT4,
                "WqT4": WqT4,
                "wa_pad": wa_pad,
                "sel": sel,
                "ba4": ba4,
                "diag4": diag4,
            }
        )
    return in_maps


def _install_profile_shim():
    """Provide antenv.axon_hooks + disable artifact upload so that
    run_bass_kernel_spmd(trace=True) can capture NTFF profiles in this
    container (the boot-time hook install is absent here)."""
    import types
    import ctypes
    import contextlib

    if "antenv.axon_hooks" not in sys.modules:
        mod = types.ModuleType("antenv.axon_hooks")
        holder = {"h": None}
        mod.set_axon_ntff_profile_hook = lambda h: holder.update(h=h)
        mod.get_axon_ntff_profile_hook = lambda: holder["h"]
        sys.modules["antenv.axon_hooks"] = mod
        import antenv

        antenv.axon_hooks = mod

    from antenv.axon_hooks import (
        get_axon_ntff_profile_hook,
        set_axon_ntff_profile_hook,
    )

    if get_axon_ntff_profile_hook() is None:
        lib = ctypes.CDLL("/opt/axon/libaxon_pjrt.so")
        if hasattr(lib, "axon_start_nrt_profile"):
            lib.axon_start_nrt_profile.argtypes = [
                ctypes.POINTER(ctypes.c_int64),
                ctypes.c_size_t,
            ]
            lib.axon_start_nrt_profile.restype = ctypes.c_int64
            lib.axon_stop_nrt_profile.argtypes = [ctypes.c_char_p]
            lib.axon_stop_nrt_profile.restype = ctypes.c_int64

            @contextlib.contextmanager
            def _hook(output_dir, device_ids):
                import jax

                jax.devices()
                if device_ids:
                    ids = (ctypes.c_int64 * len(device_ids))(*device_ids)
                    rc = lib.axon_start_nrt_profile(ids, len(device_ids))
                else:
                    rc = lib.axon_start_nrt_profile(None, 0)
                if rc != 0:
                    raise RuntimeError(f"axon_start_nrt_profile rc={rc}")
                try:
                    yield
                finally:
                    n = lib.axon_stop_nrt_profile(str(output_dir).encode())
                    print(f"profile: {n} file(s) written to {output_dir}")

            set_axon_ntff_profile_hook(_hook)

    from concourse import bass_utils as bu

    bu.upload_artifacts = lambda tmpdir: f"local:{tmpdir}"


def kernel(ctx_val, ctx_key, ctx_mask, ht_query, Wq, Wa, ba, _trace=False):
    from concourse.bass_utils import run_bass_kernel_spmd

    if _trace:
        _install_profile_shim()

    if "nc" not in _CACHE:
        _CACHE["nc"] = _build_nc()
    nc = _CACHE["nc"]

    in_maps = _prep_in_maps(ctx_val, ctx_key, ctx_mask, ht_query, Wq, Wa, ba)
    res = run_bass_kernel_spmd(
        nc, in_maps, core_ids=list(range(NCORES)), trace=_trace
    )
    if _trace:
        print(f"HW exec time: {res.exec_time_ns} ns")
        _CACHE["exec_time_ns"] = res.exec_time_ns
        _CACHE["results_obj"] = res

    ct = np.concatenate([r["out_ct"] for r in res.results], axis=0)
    s = np.concatenate([r["out_s"] for r in res.results], axis=0)
    return ct.astype(np.float32), s.reshape(B, H, W).astype(np.float32)


# revision 25
# speedup vs baseline: 1.0188x; 1.0188x over previous
"""Trainium2 Bass kernel for nn_Attention_16355235463288.

Additive attention:
    q  = ht_query @ Wq.T                      [B, D]
    e  = tanh(ctx_key + q[:, None, None, :])  [B, H, W, D]
    s  = einsum('bhwd,d->bhw', e, Wa[0]) + ba [B, H, W]
    s  = exp(s - max(s)) * mask ; s /= (sum_hw(s) + 1e-10)
    ct = einsum('bchw,bhw->bc', ctx_val, s)   [B, C]
    returns (ct, s)

Sharding: pure data parallel, B=32 over 8 cores (BL=4 per core). Params
replicated. No collectives. The global max-subtract in the reference is
dropped: softmax ratios are invariant to the subtracted constant except
through the +1e-10 term, where the relative effect is ~1e-11; |s| is
bounded by sum|Wa| ~ 20 so exp() cannot overflow in fp32. The mask is
folded in as a host-precomputed log(mask) initializer of the score
accumulator, so exp(s + logmask) = exp(s) * mask with no mask multiply.

Per-core dataflow (each stage streams ~33.5 MB from HBM), per batch b so
stage 2 of batch b overlaps stage 1 of batch b+1:
  stage 0: qT[d, b] via PE matmul from host-pretransposed WqT/htT (fp32).
  stage 1 (ctx_key):
    SWDGE cast-loads key tiles [128(hw), 512(d)] f32->bf16
    -> PE transpose 128x128 bf16 blocks into PSUM [128(d), hw]
    -> ACT tanh(psum + qT_bias) -> bf16 SBUF (fused q-add)
    -> PE matmul with zero-padded-column Wa weights -> scores [4, 512]
    -> DVE in-place add into s_sb[4, HW] (rows != b get += 0).
  softmax(b): ACT exp(s_sb + ba) -> u bf16 with accum_out giving the
    denominator for free; DVE reciprocal; tiny PE matmul broadcasts
    recip[b] over 128 partitions.
  stage 2 (ctx_val):
    PE selector matmul broadcasts u[b] over 128 partitions
    -> ACT Copy with scale=recip128 (fused normalize) -> ubc bf16
    -> SWDGE cast-load val tiles [128(c), 4096(hw)] f32->bf16
    -> DVE mul + reduce over hw -> ct column.
  tail: one fp32 exp + normalize for the s output.

tensor_tensor_reduce is avoided: it faults the NRT runtime on this
fleet (bisected 2026-08-03); plain tensor_tensor + tensor_reduce works.
"""

import sys
import numpy as np

for _p in ("/opt/trn_rl_repo", "/opt/pypackages"):
    if _p not in sys.path:
        sys.path.append(_p)

B, H, W, D, N, C = 32, 64, 64, 512, 512, 512
NCORES = 8
BL = B // NCORES          # 4 batches per core
HW = H * W                # 4096
G = 1024                  # hw positions per stage-1 group
NG = HW // G              # 4 groups
TPG = G // 128            # 8 hw-tiles per group

_CACHE = {}


def _build_nc():
    import concourse.bass as bass
    import concourse.mybir as mybir
    import concourse.tile as tile
    from concourse import bacc
    from concourse.masks import make_identity
    from contextlib import ExitStack

    f32 = mybir.dt.float32
    bf16 = mybir.dt.bfloat16
    AF = mybir.ActivationFunctionType
    OP = mybir.AluOpType

    nc = bacc.Bacc(None, target_bir_lowering=False, debug=False)

    key_d = nc.declare_dram_parameter("key", [BL, HW, D], f32, isOutput=False)
    val_d = nc.declare_dram_parameter("val", [BL, C, HW], f32, isOutput=False)
    lmask_d = nc.declare_dram_parameter("lmask", [BL, HW], f32, isOutput=False)
    htT_d = nc.declare_dram_parameter("htT4", [128, 4, BL], f32, isOutput=False)
    wqT_d = nc.declare_dram_parameter("WqT4", [128, 4, D], f32, isOutput=False)
    wap_d = nc.declare_dram_parameter("wa_pad", [128, BL, 4, BL], f32, isOutput=False)
    sel_d = nc.declare_dram_parameter("sel", [BL, BL, 128], f32, isOutput=False)
    ba_d = nc.declare_dram_parameter("ba4", [BL, 1], f32, isOutput=False)
    dg_d = nc.declare_dram_parameter("diag4", [BL, BL * 4], f32, isOutput=False)
    ct_d = nc.declare_dram_parameter("out_ct", [BL, C], f32, isOutput=True)
    s_d = nc.declare_dram_parameter("out_s", [BL, HW], f32, isOutput=True)

    with tile.TileContext(nc) as tc, ExitStack() as ctx:
        const = ctx.enter_context(tc.tile_pool(name="const", bufs=1))
        keyp = ctx.enter_context(tc.tile_pool(name="keyp", bufs=4))
        eTp = ctx.enter_context(tc.tile_pool(name="eTp", bufs=2))
        valp = ctx.enter_context(tc.tile_pool(name="valp", bufs=10))
        sbcp = ctx.enter_context(tc.tile_pool(name="sbcp", bufs=2))
        scrp = ctx.enter_context(tc.tile_pool(name="scrp", bufs=2))
        smallp = ctx.enter_context(tc.tile_pool(name="smallp", bufs=1))
        up = ctx.enter_context(tc.tile_pool(name="up", bufs=2))
        psT = ctx.enter_context(tc.tile_pool(name="psT", bufs=2, space="PSUM"))
        psS = ctx.enter_context(tc.tile_pool(name="psS", bufs=2, space="PSUM"))
        psB = ctx.enter_context(tc.tile_pool(name="psB", bufs=2, space="PSUM"))

        identity = const.tile([128, 128], bf16)
        make_identity(nc, identity)
        htT = const.tile([128, 4, BL], f32)
        nc.sync.dma_start(htT[:], htT_d[:])
        wqT = const.tile([128, 4, D], f32)
        nc.sync.dma_start(wqT[:], wqT_d[:])
        wap = const.tile([128, BL, 4, BL], bf16)
        nc.gpsimd.dma_start(wap[:], wap_d[:])
        sel = const.tile([BL, BL, 128], bf16)
        nc.gpsimd.dma_start(sel[:], sel_d[:])
        ba4 = const.tile([BL, 1], f32)
        nc.sync.dma_start(ba4[:], ba_d[:])
        diag4 = const.tile([BL, BL * 4], f32)
        nc.sync.dma_start(diag4[:], dg_d[:])
        ones4 = const.tile([BL, 128], f32)
        nc.vector.memset(ones4[:], 1.0)
        ct_sb = const.tile([128, BL * 4], f32)
        ctp = const.tile([128, BL * 4, 2], f32)

        # stage 0: qT[d_local, (k, b)] = sum_n Wq[d, n] * ht[b, n]
        ps_q = psS.tile([128, 4 * BL], f32, tag="pss")
        for k in range(4):
            for nk in range(4):
                nc.tensor.matmul(
                    ps_q[:, k * BL:(k + 1) * BL],
                    wqT[:, nk, k * 128:(k + 1) * 128],
                    htT[:, nk, :],
                    start=(nk == 0),
                    stop=(nk == 3),
                )
        qT = const.tile([128, 4 * BL], f32)
        nc.vector.tensor_copy(qT[:], ps_q[:])

        # score accumulator, initialized with log(mask) so the mask is
        # applied for free by the exp
        s_sb = smallp.tile([BL, HW], f32, tag="s_sb")
        nc.sync.dma_start(s_sb[:], lmask_d[:])
        denom = smallp.tile([BL, 1], f32, tag="denom")
        recip = smallp.tile([BL, 1], f32, tag="recip")

        HH = HW // 2  # 2048, half of the hw axis
        for b in range(BL):
            val_tiles = {}
            for half in range(2):
                # ---- stage 1 for this half (2 groups of G=1024) ----
                for gi in range(2):
                    g = half * 2 + gi
                    key_sb = keyp.tile([128, TPG, D], bf16, tag="key")
                    nc.gpsimd.dma_start(
                        key_sb[:],
                        key_d[b, g * G:(g + 1) * G, :].rearrange(
                            "(t p) d -> p t d", p=128
                        ),
                    )
                    # prefetch val halves: 2 per group -> 8 per batch
                    for vi in range(2):
                        cc = gi * 2 + vi
                        vt = valp.tile(
                            [128, HH], bf16, tag="val",
                            name=f"val_{b}_{half}_{cc}",
                        )
                        nc.gpsimd.dma_start(
                            vt[:],
                            val_d[b, cc * 128:(cc + 1) * 128,
                                  half * HH:(half + 1) * HH],
                        )
                        val_tiles[(half, cc)] = vt
                    eT = eTp.tile([128, 4, G], bf16, tag="eT")
                    for k in range(4):
                        psT_t = psT.tile([128, G], bf16, tag="pst")
                        for t in range(TPG):
                            nc.tensor.transpose(
                                psT_t[:, t * 128:(t + 1) * 128],
                                key_sb[:, t, k * 128:(k + 1) * 128],
                                identity,
                            )
                        nc.scalar.activation(
                            eT[:, k, :],
                            psT_t[:],
                            AF.Tanh,
                            bias=qT[:, (k * BL + b):(k * BL + b) + 1],
                        )
                    for c in range(G // 512):
                        ps_s = psS.tile([BL, 512], f32, tag="pss")
                        for k in range(4):
                            nc.tensor.matmul(
                                ps_s[:],
                                wap[:, b, k, :],
                                eT[:, k, c * 512:(c + 1) * 512],
                                start=(k == 0),
                                stop=(k == 3),
                            )
                        off = g * G + c * 512
                        # rows != b of ps_s are zero (zero-padded Wa cols),
                        # so an in-place add deposits row b w/o clobbering
                        nc.vector.tensor_tensor(
                            s_sb[:, off:off + 512],
                            s_sb[:, off:off + 512],
                            ps_s[:],
                            OP.add,
                        )

                # ---- unnormalized softmax numerator for this half ----
                # (rows != b hold junk masked by the selector matmul)
                u = up.tile([BL, HH], bf16, tag="u")
                nc.scalar.activation(
                    u[:], s_sb[:, half * HH:(half + 1) * HH],
                    AF.Exp, bias=ba4[:],
                )
                sbc = sbcp.tile([128, HH], bf16, tag="sbc")
                for h2 in range(2):
                    ps_b = psB.tile([128, 1024], f32, tag="psb")
                    for j in range(2):
                        off = h2 * 1024 + j * 512
                        nc.tensor.matmul(
                            ps_b[:, j * 512:(j + 1) * 512],
                            sel[:, b, :],
                            u[:, off:off + 512],
                            start=True,
                            stop=True,
                        )
                    nc.scalar.activation(
                        sbc[:, h2 * 1024:(h2 + 1) * 1024], ps_b[:], AF.Copy
                    )

                # ---- stage 2 partial reductions for this half ----
                for cc in range(4):
                    val_sb = val_tiles[(half, cc)]
                    prod = scrp.tile([128, HH], bf16, tag="prod")
                    sum2 = scrp.tile([128, HH // 2], bf16, tag="sum2")
                    idx = b * 4 + cc
                    eng = nc.vector if cc % 2 == 0 else nc.gpsimd
                    eng.tensor_tensor(
                        prod[:], val_sb[:], sbc[:], OP.mult
                    )
                    nc.vector.tensor_tensor(
                        sum2[:], prod[:, 0:HH // 2], prod[:, HH // 2:HH],
                        OP.add,
                    )
                    nc.vector.tensor_reduce(
                        ctp[:, idx, half:half + 1],
                        sum2[:],
                        mybir.AxisListType.X,
                        OP.add,
                    )

        # ---- tail: denominators, ct normalize, then the s output ----
        s_e = smallp.tile([BL, HW], f32, tag="s_e")
        exp_tail = nc.scalar.activation(
            s_e[:], s_sb[:], AF.Exp, bias=ba4[:], accum_out=denom[:]
        )
        nc.vector.reciprocal(recip[:], denom[:])
        comb = nc.vector.tensor_tensor(
            ct_sb[:], ctp[:, :, 0], ctp[:, :, 1], OP.add
        )
        # keep the big fp32 exp out of the last batch's critical window
        from concourse.tile import add_dep_helper as _adh
        _adh(exp_tail.ins, comb.ins, sync=True, reason="defer s-tail exp")

        # normalize ct: recip_cols[p, (b,cc)] = recip[b]
        rdiag = smallp.tile([BL, BL * 4], f32, tag="rdiag")
        nc.vector.tensor_scalar_mul(rdiag[:], diag4[:], recip[:])
        ps_n = psB.tile([128, 1024], f32, tag="psb")
        nc.tensor.matmul(
            ps_n[:, 0:BL * 4], ones4[:], rdiag[:], start=True, stop=True
        )
        nc.vector.tensor_tensor(
            ct_sb[:], ct_sb[:], ps_n[:, 0:BL * 4], OP.mult
        )
        nc.sync.dma_start(
            ct_d.rearrange("b (c p) -> p b c", p=128),
            ct_sb.rearrange("p (b c) -> p b c", b=BL),
        )
        nc.vector.tensor_scalar_mul(s_sb[:], s_e[:], recip[:])
        nc.sync.dma_start(s_d[:], s_sb[:])

    nc.compile()
    return nc


def _prep_in_maps(ctx_val, ctx_key, ctx_mask, ht_query, Wq, Wa, ba):
    key = np.ascontiguousarray(ctx_key.reshape(B, HW, D), dtype=np.float32)
    val = np.ascontiguousarray(ctx_val.reshape(B, C, HW), dtype=np.float32)
    mask = np.asarray(ctx_mask, dtype=np.float32).reshape(B, HW)
    with np.errstate(divide="ignore"):
        lmask = np.where(mask > 0, np.log(np.maximum(mask, 1e-38)), -1e30)
    lmask = np.ascontiguousarray(lmask, dtype=np.float32)
    ht = np.asarray(ht_query, dtype=np.float32)
    Wq = np.asarray(Wq, dtype=np.float32)
    Wa = np.asarray(Wa, dtype=np.float32)
    ba = np.asarray(ba, dtype=np.float32)

    # WqT4[n_local, nk, d] = Wq[d, nk*128 + n_local]
    WqT4 = np.ascontiguousarray(
        Wq.T.reshape(4, 128, D).transpose(1, 0, 2), dtype=np.float32
    )
    # wa_pad[d_local, b, k, col] = Wa[0, k*128+d_local] if col == b else 0
    waT = Wa[0].reshape(4, 128).T  # [d_local, k]
    wa_pad = np.zeros((128, BL, 4, BL), dtype=np.float32)
    for b in range(BL):
        wa_pad[:, b, :, b] = waT
    # sel[b, p, m] = 1 if p == b
    sel = np.zeros((BL, BL, 128), dtype=np.float32)
    for b in range(BL):
        sel[b, b, :] = 1.0
    ba4 = np.full((BL, 1), float(ba[0]), dtype=np.float32)
    # diag4[p, (b, cc)] = 1 if p == b (for the end-of-kernel ct normalize)
    diag4 = np.zeros((BL, BL * 4), dtype=np.float32)
    for b in range(BL):
        diag4[b, b * 4:(b + 1) * 4] = 1.0

    in_maps = []
    for core in range(NCORES):
        sl = slice(core * BL, (core + 1) * BL)
        ht_sh = ht[sl]  # [BL, N]
        # htT4[n_local, nk, b] = ht_sh[b, nk*128 + n_local]
        htT4 = np.ascontiguousarray(
            ht_sh.T.reshape(4, 128, BL).transpose(1, 0, 2), dtype=np.float32
        )
        in_maps.append(
            {
                "key": key[sl],
                "val": val[sl],
                "lmask": lmask[sl],
                "htT4": htT4,
                "WqT4": WqT4,
                "wa_pad": wa_pad,
                "sel": sel,
                "ba4": ba4,
                "diag4": diag4,
            }
        )
    return in_maps


def _install_profile_shim():
    """Provide antenv.axon_hooks + disable artifact upload so that
    run_bass_kernel_spmd(trace=True) can capture NTFF profiles in this
    container (the boot-time hook install is absent here)."""
    import types
    import ctypes
    import contextlib

    if "antenv.axon_hooks" not in sys.modules:
        mod = types.ModuleType("antenv.axon_hooks")
        holder = {"h": None}
        mod.set_axon_ntff_profile_hook = lambda h: holder.update(h=h)
        mod.get_axon_ntff_profile_hook = lambda: holder["h"]
        sys.modules["antenv.axon_hooks"] = mod
        import antenv

        antenv.axon_hooks = mod

    from antenv.axon_hooks import (
        get_axon_ntff_profile_hook,
        set_axon_ntff_profile_hook,
    )

    if get_axon_ntff_profile_hook() is None:
        lib = ctypes.CDLL("/opt/axon/libaxon_pjrt.so")
        if hasattr(lib, "axon_start_nrt_profile"):
            lib.axon_start_nrt_profile.argtypes = [
                ctypes.POINTER(ctypes.c_int64),
                ctypes.c_size_t,
            ]
            lib.axon_start_nrt_profile.restype = ctypes.c_int64
            lib.axon_stop_nrt_profile.argtypes = [ctypes.c_char_p]
            lib.axon_stop_nrt_profile.restype = ctypes.c_int64

            @contextlib.contextmanager
            def _hook(output_dir, device_ids):
                import jax

                jax.devices()
                if device_ids:
                    ids = (ctypes.c_int64 * len(device_ids))(*device_ids)
                    rc = lib.axon_start_nrt_profile(ids, len(device_ids))
                else:
                    rc = lib.axon_start_nrt_profile(None, 0)
                if rc != 0:
                    raise RuntimeError(f"axon_start_nrt_profile rc={rc}")
                try:
                    yield
                finally:
                    n = lib.axon_stop_nrt_profile(str(output_dir).encode())
                    print(f"profile: {n} file(s) written to {output_dir}")

            set_axon_ntff_profile_hook(_hook)

    from concourse import bass_utils as bu

    bu.upload_artifacts = lambda tmpdir: f"local:{tmpdir}"


def kernel(ctx_val, ctx_key, ctx_mask, ht_query, Wq, Wa, ba, _trace=False):
    from concourse.bass_utils import run_bass_kernel_spmd

    if _trace:
        _install_profile_shim()

    if "nc" not in _CACHE:
        _CACHE["nc"] = _build_nc()
    nc = _CACHE["nc"]

    in_maps = _prep_in_maps(ctx_val, ctx_key, ctx_mask, ht_query, Wq, Wa, ba)
    res = run_bass_kernel_spmd(
        nc, in_maps, core_ids=list(range(NCORES)), trace=_trace
    )
    if _trace:
        print(f"HW exec time: {res.exec_time_ns} ns")
        _CACHE["exec_time_ns"] = res.exec_time_ns
        _CACHE["results_obj"] = res

    ct = np.concatenate([r["out_ct"] for r in res.results], axis=0)
    s = np.concatenate([r["out_s"] for r in res.results], axis=0)
    return ct.astype(np.float32), s.reshape(B, H, W).astype(np.float32)


# revision 30
# speedup vs baseline: 1.0282x; 1.0092x over previous
"""Trainium2 Bass kernel for nn_Attention_16355235463288.

Additive attention:
    q  = ht_query @ Wq.T                      [B, D]
    e  = tanh(ctx_key + q[:, None, None, :])  [B, H, W, D]
    s  = einsum('bhwd,d->bhw', e, Wa[0]) + ba [B, H, W]
    s  = exp(s - max(s)) * mask ; s /= (sum_hw(s) + 1e-10)
    ct = einsum('bchw,bhw->bc', ctx_val, s)   [B, C]
    returns (ct, s)

Sharding: pure data parallel, B=32 over 8 cores (BL=4 per core). Params
replicated. No collectives. The global max-subtract in the reference is
dropped: softmax ratios are invariant to the subtracted constant except
through the +1e-10 term, where the relative effect is ~1e-11; |s| is
bounded by sum|Wa| ~ 20 so exp() cannot overflow in fp32. The mask is
folded in as a host-precomputed log(mask) initializer of the score
accumulator, so exp(s + logmask) = exp(s) * mask with no mask multiply.

Per-core dataflow (each stage streams ~33.5 MB from HBM), per batch b so
stage 2 of batch b overlaps stage 1 of batch b+1:
  stage 0: qT[d, b] via PE matmul from host-pretransposed WqT/htT (fp32).
  stage 1 (ctx_key):
    SWDGE cast-loads key tiles [128(hw), 512(d)] f32->bf16
    -> PE transpose 128x128 bf16 blocks into PSUM [128(d), hw]
    -> ACT tanh(psum + qT_bias) -> bf16 SBUF (fused q-add)
    -> PE matmul with zero-padded-column Wa weights -> scores [4, 512]
    -> DVE in-place add into s_sb[4, HW] (rows != b get += 0).
  softmax(b): ACT exp(s_sb + ba) -> u bf16 with accum_out giving the
    denominator for free; DVE reciprocal; tiny PE matmul broadcasts
    recip[b] over 128 partitions.
  stage 2 (ctx_val):
    PE selector matmul broadcasts u[b] over 128 partitions
    -> ACT Copy with scale=recip128 (fused normalize) -> ubc bf16
    -> SWDGE cast-load val tiles [128(c), 4096(hw)] f32->bf16
    -> DVE mul + reduce over hw -> ct column.
  tail: one fp32 exp + normalize for the s output.

tensor_tensor_reduce is avoided: it faults the NRT runtime on this
fleet (bisected 2026-08-03); plain tensor_tensor + tensor_reduce works.
"""

import sys
import numpy as np

for _p in ("/opt/trn_rl_repo", "/opt/pypackages"):
    if _p not in sys.path:
        sys.path.append(_p)

B, H, W, D, N, C = 32, 64, 64, 512, 512, 512
NCORES = 8
BL = B // NCORES          # 4 batches per core
HW = H * W                # 4096
G = 1024                  # hw positions per stage-1 group
NG = HW // G              # 4 groups
TPG = G // 128            # 8 hw-tiles per group

_CACHE = {}


def _build_nc():
    import concourse.bass as bass
    import concourse.mybir as mybir
    import concourse.tile as tile
    from concourse import bacc
    from concourse.masks import make_identity
    from contextlib import ExitStack

    f32 = mybir.dt.float32
    bf16 = mybir.dt.bfloat16
    AF = mybir.ActivationFunctionType
    OP = mybir.AluOpType

    nc = bacc.Bacc(None, target_bir_lowering=False, debug=False)

    key_d = nc.declare_dram_parameter("key", [BL, HW, D], f32, isOutput=False)
    val_d = nc.declare_dram_parameter("val", [BL, C, HW], f32, isOutput=False)
    lmask_d = nc.declare_dram_parameter("lmask", [BL, HW], f32, isOutput=False)
    htT_d = nc.declare_dram_parameter("htT4", [128, 4, BL], f32, isOutput=False)
    wqT_d = nc.declare_dram_parameter("WqT4", [128, 4, D], f32, isOutput=False)
    wap_d = nc.declare_dram_parameter("wa_pad", [128, BL, 4, BL], f32, isOutput=False)
    sel_d = nc.declare_dram_parameter("sel", [BL, BL, 128], f32, isOutput=False)
    ba_d = nc.declare_dram_parameter("ba4", [BL, 1], f32, isOutput=False)
    dg_d = nc.declare_dram_parameter("diag4", [BL, BL * 4], f32, isOutput=False)
    c4_d = nc.declare_dram_parameter("cols4", [1, BL, BL], f32, isOutput=False)
    bm_d = nc.declare_dram_parameter("bmask", [BL, BL * 8], f32, isOutput=False)
    ct_d = nc.declare_dram_parameter("out_ct", [BL, C], f32, isOutput=True)
    s_d = nc.declare_dram_parameter("out_s", [BL, HW], f32, isOutput=True)

    with tile.TileContext(nc) as tc, ExitStack() as ctx:
        const = ctx.enter_context(tc.tile_pool(name="const", bufs=1))
        keyp = ctx.enter_context(tc.tile_pool(name="keyp", bufs=4))
        eTp = ctx.enter_context(tc.tile_pool(name="eTp", bufs=2))
        valp = ctx.enter_context(tc.tile_pool(name="valp", bufs=10))
        sbcp = ctx.enter_context(tc.tile_pool(name="sbcp", bufs=2))
        scrp = ctx.enter_context(tc.tile_pool(name="scrp", bufs=2))
        smallp = ctx.enter_context(tc.tile_pool(name="smallp", bufs=1))
        up = ctx.enter_context(tc.tile_pool(name="up", bufs=2))
        psT = ctx.enter_context(tc.tile_pool(name="psT", bufs=2, space="PSUM"))
        psS = ctx.enter_context(tc.tile_pool(name="psS", bufs=2, space="PSUM"))
        psB = ctx.enter_context(tc.tile_pool(name="psB", bufs=2, space="PSUM"))

        identity = const.tile([128, 128], bf16)
        make_identity(nc, identity)
        htT = const.tile([128, 4, BL], f32)
        nc.sync.dma_start(htT[:], htT_d[:])
        wqT = const.tile([128, 4, D], f32)
        nc.sync.dma_start(wqT[:], wqT_d[:])
        wap = const.tile([128, BL, 4, BL], bf16)
        nc.gpsimd.dma_start(wap[:], wap_d[:])
        sel = const.tile([BL, BL, 128], bf16)
        nc.gpsimd.dma_start(sel[:], sel_d[:])
        ba4 = const.tile([BL, 1], f32)
        nc.sync.dma_start(ba4[:], ba_d[:])
        diag4 = const.tile([BL, BL * 4], f32)
        nc.sync.dma_start(diag4[:], dg_d[:])
        ones4 = const.tile([BL, 128], f32)
        nc.vector.memset(ones4[:], 1.0)
        cols4 = const.tile([1, BL, BL], bf16)
        nc.gpsimd.dma_start(cols4[:], c4_d[:])
        bmask = const.tile([BL, BL * 8], f32)
        nc.sync.dma_start(bmask[:], bm_d[:])
        lmask1 = const.tile([1, BL * HW], bf16)
        nc.gpsimd.dma_start(lmask1[:], lmask_d[:].rearrange("b h -> (b h)"))
        ct_sb = const.tile([128, BL * 4], f32)
        ctp = const.tile([128, BL * 4, 2], f32)

        # stage 0: qT[d_local, (k, b)] = sum_n Wq[d, n] * ht[b, n]
        ps_q = psS.tile([128, 4 * BL], f32, tag="pss")
        for k in range(4):
            for nk in range(4):
                nc.tensor.matmul(
                    ps_q[:, k * BL:(k + 1) * BL],
                    wqT[:, nk, k * 128:(k + 1) * 128],
                    htT[:, nk, :],
                    start=(nk == 0),
                    stop=(nk == 3),
                )
        qT = const.tile([128, 4 * BL], f32)
        nc.vector.tensor_copy(qT[:], ps_q[:])

        denom = smallp.tile([BL, 1], f32, tag="denom")
        recip = smallp.tile([BL, 1], f32, tag="recip")
        den_all = smallp.tile([BL, BL * 8], f32, tag="den_all")
        u_bt = [
            smallp.tile([BL, HW], bf16, tag=f"u{b}", name=f"u_{b}")
            for b in range(BL)
        ]

        HH = HW // 2  # 2048, half of the hw axis
        for b in range(BL):
            val_tiles = {}
            for half in range(2):
                # ---- stage 1 for this half (2 groups of G=1024) ----
                for gi in range(2):
                    g = half * 2 + gi
                    key_sb = keyp.tile([128, TPG, D], bf16, tag="key")
                    nc.gpsimd.dma_start(
                        key_sb[:],
                        key_d[b, g * G:(g + 1) * G, :].rearrange(
                            "(t p) d -> p t d", p=128
                        ),
                    )
                    # prefetch val halves: 2 per group -> 8 per batch
                    for vi in range(2):
                        cc = gi * 2 + vi
                        vt = valp.tile(
                            [128, HH], bf16, tag="val",
                            name=f"val_{b}_{half}_{cc}",
                        )
                        nc.gpsimd.dma_start(
                            vt[:],
                            val_d[b, cc * 128:(cc + 1) * 128,
                                  half * HH:(half + 1) * HH],
                        )
                        val_tiles[(half, cc)] = vt
                    eT = eTp.tile([128, 4, G], bf16, tag="eT")
                    for k in range(4):
                        psT_t = psT.tile([128, G], bf16, tag="pst")
                        for t in range(TPG):
                            nc.tensor.transpose(
                                psT_t[:, t * 128:(t + 1) * 128],
                                key_sb[:, t, k * 128:(k + 1) * 128],
                                identity,
                            )
                        nc.scalar.activation(
                            eT[:, k, :],
                            psT_t[:],
                            AF.Tanh,
                            bias=qT[:, (k * BL + b):(k * BL + b) + 1],
                        )
                    for c in range(G // 512):
                        ps_s = psS.tile([BL, 512], f32, tag="pss")
                        for k in range(4):
                            nc.tensor.matmul(
                                ps_s[:],
                                wap[:, b, k, :],
                                eT[:, k, c * 512:(c + 1) * 512],
                                start=(k == 0),
                                stop=False,
                            )
                        off = g * G + c * 512
                        # accumulate log(mask) into row b (K=1 matmul)
                        nc.tensor.matmul(
                            ps_s[:],
                            cols4[:, b, :],
                            lmask1[:, b * HW + off:b * HW + off + 512],
                            start=False,
                            stop=True,
                        )
                        # exp straight off PSUM; rows != b are junk that
                        # the downstream selector matmuls zero out
                        ci = b * 8 + g * 2 + c
                        nc.scalar.activation(
                            u_bt[b][:, off:off + 512],
                            ps_s[:],
                            AF.Exp,
                            bias=ba4[:],
                            accum_out=den_all[:, ci:ci + 1],
                        )

                # ---- broadcast the (unnormalized) numerator ----
                sbc = sbcp.tile([128, HH], bf16, tag="sbc")
                for h2 in range(2):
                    ps_b = psB.tile([128, 1024], f32, tag="psb")
                    for j in range(2):
                        off = half * HH + h2 * 1024 + j * 512
                        nc.tensor.matmul(
                            ps_b[:, j * 512:(j + 1) * 512],
                            sel[:, b, :],
                            u_bt[b][:, off:off + 512],
                            start=True,
                            stop=True,
                        )
                    nc.scalar.activation(
                        sbc[:, h2 * 1024:(h2 + 1) * 1024], ps_b[:], AF.Copy
                    )

                # ---- stage 2 partial reductions for this half ----
                for cc in range(4):
                    val_sb = val_tiles[(half, cc)]
                    prod = scrp.tile([128, HH], bf16, tag="prod")
                    sum2 = scrp.tile([128, HH // 2], bf16, tag="sum2")
                    idx = b * 4 + cc
                    nc.vector.tensor_tensor(
                        prod[:], val_sb[:], sbc[:], OP.mult
                    )
                    nc.vector.tensor_tensor(
                        sum2[:], prod[:, 0:HH // 2], prod[:, HH // 2:HH],
                        OP.add,
                    )
                    nc.vector.tensor_reduce(
                        ctp[:, idx, half:half + 1],
                        sum2[:],
                        mybir.AxisListType.X,
                        OP.add,
                    )

        # ---- tail: denominators, ct normalize, then the s output ----
        den_m = smallp.tile([BL, BL * 8], f32, tag="den_m")
        nc.vector.tensor_tensor(den_m[:], den_all[:], bmask[:], OP.mult)
        nc.vector.tensor_reduce(
            denom[:], den_m[:], mybir.AxisListType.X, OP.add
        )
        den2 = smallp.tile([BL, 1], f32, tag="den2")
        nc.vector.tensor_scalar_add(den2[:], denom[:], 1e-10)
        nc.vector.reciprocal(recip[:], den2[:])
        nc.vector.tensor_tensor(
            ct_sb[:], ctp[:, :, 0], ctp[:, :, 1], OP.add
        )

        # normalize ct: recip_cols[p, (b,cc)] = recip[b]
        rdiag = smallp.tile([BL, BL * 4], f32, tag="rdiag")
        nc.vector.tensor_scalar_mul(rdiag[:], diag4[:], recip[:])
        ps_n = psB.tile([128, 1024], f32, tag="psb")
        nc.tensor.matmul(
            ps_n[:, 0:BL * 4], ones4[:], rdiag[:], start=True, stop=True
        )
        nc.vector.tensor_tensor(
            ct_sb[:], ct_sb[:], ps_n[:, 0:BL * 4], OP.mult
        )
        nc.sync.dma_start(
            ct_d.rearrange("b (c p) -> p b c", p=128),
            ct_sb.rearrange("p (b c) -> p b c", b=BL),
        )

        # s output: normalize each batch's numerator, DMA row b (bf16->f32)
        for b in range(BL):
            un = up.tile([BL, HW], bf16, tag="un")
            nc.vector.tensor_scalar_mul(un[:], u_bt[b][:], recip[:])
            nc.gpsimd.dma_start(s_d[b:b + 1, :], un[b:b + 1, :])

    nc.compile()
    return nc


def _prep_in_maps(ctx_val, ctx_key, ctx_mask, ht_query, Wq, Wa, ba):
    key = np.ascontiguousarray(ctx_key.reshape(B, HW, D), dtype=np.float32)
    val = np.ascontiguousarray(ctx_val.reshape(B, C, HW), dtype=np.float32)
    mask = np.asarray(ctx_mask, dtype=np.float32).reshape(B, HW)
    with np.errstate(divide="ignore"):
        lmask = np.where(mask > 0, np.log(np.maximum(mask, 1e-38)), -1e30)
    lmask = np.ascontiguousarray(lmask, dtype=np.float32)
    ht = np.asarray(ht_query, dtype=np.float32)
    Wq = np.asarray(Wq, dtype=np.float32)
    Wa = np.asarray(Wa, dtype=np.float32)
    ba = np.asarray(ba, dtype=np.float32)

    # WqT4[n_local, nk, d] = Wq[d, nk*128 + n_local]
    WqT4 = np.ascontiguousarray(
        Wq.T.reshape(4, 128, D).transpose(1, 0, 2), dtype=np.float32
    )
    # wa_pad[d_local, b, k, col] = Wa[0, k*128+d_local] if col == b else 0
    waT = Wa[0].reshape(4, 128).T  # [d_local, k]
    wa_pad = np.zeros((128, BL, 4, BL), dtype=np.float32)
    for b in range(BL):
        wa_pad[:, b, :, b] = waT
    # sel[b, p, m] = 1 if p == b
    sel = np.zeros((BL, BL, 128), dtype=np.float32)
    for b in range(BL):
        sel[b, b, :] = 1.0
    ba4 = np.full((BL, 1), float(ba[0]), dtype=np.float32)
    # diag4[p, (b, cc)] = 1 if p == b (for the end-of-kernel ct normalize)
    diag4 = np.zeros((BL, BL * 4), dtype=np.float32)
    for b in range(BL):
        diag4[b, b * 4:(b + 1) * 4] = 1.0
    # cols4[0, b, j] = 1 if j == b (routes the logmask row into score row b)
    cols4 = np.zeros((1, BL, BL), dtype=np.float32)
    for b in range(BL):
        cols4[0, b, b] = 1.0
    # bmask[r, (b, gc)] = 1 if r == b (extracts valid chunk denominators)
    bmask = np.zeros((BL, BL * 8), dtype=np.float32)
    for b in range(BL):
        bmask[b, b * 8:(b + 1) * 8] = 1.0

    in_maps = []
    for core in range(NCORES):
        sl = slice(core * BL, (core + 1) * BL)
        ht_sh = ht[sl]  # [BL, N]
        # htT4[n_local, nk, b] = ht_sh[b, nk*128 + n_local]
        htT4 = np.ascontiguousarray(
            ht_sh.T.reshape(4, 128, BL).transpose(1, 0, 2), dtype=np.float32
        )
        in_maps.append(
            {
                "key": key[sl],
                "val": val[sl],
                "lmask": lmask[sl],
                "htT4": htT4,
                "WqT4": WqT4,
                "wa_pad": wa_pad,
                "sel": sel,
                "ba4": ba4,
                "diag4": diag4,
                "cols4": cols4,
                "bmask": bmask,
            }
        )
    return in_maps


def _install_profile_shim():
    """Provide antenv.axon_hooks + disable artifact upload so that
    run_bass_kernel_spmd(trace=True) can capture NTFF profiles in this
    container (the boot-time hook install is absent here)."""
    import types
    import ctypes
    import contextlib

    if "antenv.axon_hooks" not in sys.modules:
        mod = types.ModuleType("antenv.axon_hooks")
        holder = {"h": None}
        mod.set_axon_ntff_profile_hook = lambda h: holder.update(h=h)
        mod.get_axon_ntff_profile_hook = lambda: holder["h"]
        sys.modules["antenv.axon_hooks"] = mod
        import antenv

        antenv.axon_hooks = mod

    from antenv.axon_hooks import (
        get_axon_ntff_profile_hook,
        set_axon_ntff_profile_hook,
    )

    if get_axon_ntff_profile_hook() is None:
        lib = ctypes.CDLL("/opt/axon/libaxon_pjrt.so")
        if hasattr(lib, "axon_start_nrt_profile"):
            lib.axon_start_nrt_profile.argtypes = [
                ctypes.POINTER(ctypes.c_int64),
                ctypes.c_size_t,
            ]
            lib.axon_start_nrt_profile.restype = ctypes.c_int64
            lib.axon_stop_nrt_profile.argtypes = [ctypes.c_char_p]
            lib.axon_stop_nrt_profile.restype = ctypes.c_int64

            @contextlib.contextmanager
            def _hook(output_dir, device_ids):
                import jax

                jax.devices()
                if device_ids:
                    ids = (ctypes.c_int64 * len(device_ids))(*device_ids)
                    rc = lib.axon_start_nrt_profile(ids, len(device_ids))
                else:
                    rc = lib.axon_start_nrt_profile(None, 0)
                if rc != 0:
                    raise RuntimeError(f"axon_start_nrt_profile rc={rc}")
                try:
                    yield
                finally:
                    n = lib.axon_stop_nrt_profile(str(output_dir).encode())
                    print(f"profile: {n} file(s) written to {output_dir}")

            set_axon_ntff_profile_hook(_hook)

    from concourse import bass_utils as bu

    bu.upload_artifacts = lambda tmpdir: f"local:{tmpdir}"


def kernel(ctx_val, ctx_key, ctx_mask, ht_query, Wq, Wa, ba, _trace=False):
    from concourse.bass_utils import run_bass_kernel_spmd

    if _trace:
        _install_profile_shim()

    if "nc" not in _CACHE:
        _CACHE["nc"] = _build_nc()
    nc = _CACHE["nc"]

    in_maps = _prep_in_maps(ctx_val, ctx_key, ctx_mask, ht_query, Wq, Wa, ba)
    res = run_bass_kernel_spmd(
        nc, in_maps, core_ids=list(range(NCORES)), trace=_trace
    )
    if _trace:
        print(f"HW exec time: {res.exec_time_ns} ns")
        _CACHE["exec_time_ns"] = res.exec_time_ns
        _CACHE["results_obj"] = res

    ct = np.concatenate([r["out_ct"] for r in res.results], axis=0)
    s = np.concatenate([r["out_s"] for r in res.results], axis=0)
    return ct.astype(np.float32), s.reshape(B, H, W).astype(np.float32)


# revision 31
# speedup vs baseline: 1.0723x; 1.0429x over previous
"""Trainium2 Bass kernel for nn_Attention_16355235463288.

Additive attention:
    q  = ht_query @ Wq.T                      [B, D]
    e  = tanh(ctx_key + q[:, None, None, :])  [B, H, W, D]
    s  = einsum('bhwd,d->bhw', e, Wa[0]) + ba [B, H, W]
    s  = exp(s - max(s)) * mask ; s /= (sum_hw(s) + 1e-10)
    ct = einsum('bchw,bhw->bc', ctx_val, s)   [B, C]
    returns (ct, s)

Sharding: pure data parallel, B=32 over 8 cores (BL=4 per core). Params
replicated. No collectives. The global max-subtract in the reference is
dropped: softmax ratios are invariant to the subtracted constant except
through the +1e-10 term, where the relative effect is ~1e-11; |s| is
bounded by sum|Wa| ~ 20 so exp() cannot overflow in fp32. The mask is
folded in as a host-precomputed log(mask) initializer of the score
accumulator, so exp(s + logmask) = exp(s) * mask with no mask multiply.

Per-core dataflow (each stage streams ~33.5 MB from HBM), per batch b so
stage 2 of batch b overlaps stage 1 of batch b+1:
  stage 0: qT[d, b] via PE matmul from host-pretransposed WqT/htT (fp32).
  stage 1 (ctx_key):
    SWDGE cast-loads key tiles [128(hw), 512(d)] f32->bf16
    -> PE transpose 128x128 bf16 blocks into PSUM [128(d), hw]
    -> ACT tanh(psum + qT_bias) -> bf16 SBUF (fused q-add)
    -> PE matmul with zero-padded-column Wa weights -> scores [4, 512]
    -> DVE in-place add into s_sb[4, HW] (rows != b get += 0).
  softmax(b): ACT exp(s_sb + ba) -> u bf16 with accum_out giving the
    denominator for free; DVE reciprocal; tiny PE matmul broadcasts
    recip[b] over 128 partitions.
  stage 2 (ctx_val):
    PE selector matmul broadcasts u[b] over 128 partitions
    -> ACT Copy with scale=recip128 (fused normalize) -> ubc bf16
    -> SWDGE cast-load val tiles [128(c), 4096(hw)] f32->bf16
    -> DVE mul + reduce over hw -> ct column.
  tail: one fp32 exp + normalize for the s output.

tensor_tensor_reduce is avoided: it faults the NRT runtime on this
fleet (bisected 2026-08-03); plain tensor_tensor + tensor_reduce works.
"""

import sys
import numpy as np

for _p in ("/opt/trn_rl_repo", "/opt/pypackages"):
    if _p not in sys.path:
        sys.path.append(_p)

B, H, W, D, N, C = 32, 64, 64, 512, 512, 512
NCORES = 8
BL = B // NCORES          # 4 batches per core
HW = H * W                # 4096
G = 1024                  # hw positions per stage-1 group
NG = HW // G              # 4 groups
TPG = G // 128            # 8 hw-tiles per group

_CACHE = {}


def _build_nc():
    import concourse.bass as bass
    import concourse.mybir as mybir
    import concourse.tile as tile
    from concourse import bacc
    from concourse.masks import make_identity
    from contextlib import ExitStack

    f32 = mybir.dt.float32
    bf16 = mybir.dt.bfloat16
    AF = mybir.ActivationFunctionType
    OP = mybir.AluOpType

    nc = bacc.Bacc(None, target_bir_lowering=False, debug=False)

    key_d = nc.declare_dram_parameter("key", [BL, HW, D], f32, isOutput=False)
    val_d = nc.declare_dram_parameter("val", [BL, C, HW], f32, isOutput=False)
    lmask_d = nc.declare_dram_parameter("lmask", [BL, HW], f32, isOutput=False)
    htT_d = nc.declare_dram_parameter("htT4", [128, 4, BL], f32, isOutput=False)
    wqT_d = nc.declare_dram_parameter("WqT4", [128, 4, D], f32, isOutput=False)
    wap_d = nc.declare_dram_parameter("wa_pad", [128, BL, 4, BL], f32, isOutput=False)
    sel_d = nc.declare_dram_parameter("sel", [BL, BL, 128], f32, isOutput=False)
    ba_d = nc.declare_dram_parameter("ba4", [BL, 1], f32, isOutput=False)
    dg_d = nc.declare_dram_parameter("diag4", [BL, BL * 4], f32, isOutput=False)
    c4_d = nc.declare_dram_parameter("cols4", [1, BL, BL], f32, isOutput=False)
    bm_d = nc.declare_dram_parameter("bmask", [BL, BL * 8], f32, isOutput=False)
    ct_d = nc.declare_dram_parameter("out_ct", [BL, C], f32, isOutput=True)
    s_d = nc.declare_dram_parameter("out_s", [BL, HW], f32, isOutput=True)

    with tile.TileContext(nc) as tc, ExitStack() as ctx:
        const = ctx.enter_context(tc.tile_pool(name="const", bufs=1))
        keyp = ctx.enter_context(tc.tile_pool(name="keyp", bufs=4))
        eTp = ctx.enter_context(tc.tile_pool(name="eTp", bufs=2))
        valp = ctx.enter_context(tc.tile_pool(name="valp", bufs=10))
        sbcp = ctx.enter_context(tc.tile_pool(name="sbcp", bufs=2))
        scrp = ctx.enter_context(tc.tile_pool(name="scrp", bufs=2))
        smallp = ctx.enter_context(tc.tile_pool(name="smallp", bufs=1))
        up = ctx.enter_context(tc.tile_pool(name="up", bufs=2))
        psT = ctx.enter_context(tc.tile_pool(name="psT", bufs=2, space="PSUM"))
        psS = ctx.enter_context(tc.tile_pool(name="psS", bufs=2, space="PSUM"))
        psB = ctx.enter_context(tc.tile_pool(name="psB", bufs=2, space="PSUM"))

        identity = const.tile([128, 128], bf16)
        make_identity(nc, identity)
        htT = const.tile([128, 4, BL], f32)
        nc.sync.dma_start(htT[:], htT_d[:])
        wqT = const.tile([128, 4, D], f32)
        nc.sync.dma_start(wqT[:], wqT_d[:])
        wap = const.tile([128, BL, 4, BL], bf16)
        nc.gpsimd.dma_start(wap[:], wap_d[:])
        sel = const.tile([BL, BL, 128], bf16)
        nc.gpsimd.dma_start(sel[:], sel_d[:])
        ba4 = const.tile([BL, 1], f32)
        nc.sync.dma_start(ba4[:], ba_d[:])
        diag4 = const.tile([BL, BL * 4], f32)
        nc.sync.dma_start(diag4[:], dg_d[:])
        ones4 = const.tile([BL, 128], f32)
        nc.vector.memset(ones4[:], 1.0)
        cols4 = const.tile([1, BL, BL], bf16)
        nc.gpsimd.dma_start(cols4[:], c4_d[:])
        bmask = const.tile([BL, BL * 8], f32)
        nc.sync.dma_start(bmask[:], bm_d[:])
        lmask1 = const.tile([1, BL * HW], bf16)
        nc.gpsimd.dma_start(lmask1[:], lmask_d[:].rearrange("b h -> (b h)"))
        ct_sb = const.tile([128, BL * 4], f32)
        ctp = const.tile([128, BL * 4, 2], f32)

        # stage 0: qT[d_local, (k, b)] = sum_n Wq[d, n] * ht[b, n]
        ps_q = psS.tile([128, 4 * BL], f32, tag="pss")
        for k in range(4):
            for nk in range(4):
                nc.tensor.matmul(
                    ps_q[:, k * BL:(k + 1) * BL],
                    wqT[:, nk, k * 128:(k + 1) * 128],
                    htT[:, nk, :],
                    start=(nk == 0),
                    stop=(nk == 3),
                )
        qT = const.tile([128, 4 * BL], f32)
        nc.vector.tensor_copy(qT[:], ps_q[:])

        denom = smallp.tile([BL, 1], f32, tag="denom")
        recip = smallp.tile([BL, 1], f32, tag="recip")
        den_all = smallp.tile([BL, BL * 8], f32, tag="den_all")
        rdiag = smallp.tile([BL, BL * 4], f32, tag="rdiag")
        u_bt = [
            smallp.tile([BL, HW], bf16, tag=f"u{b}", name=f"u_{b}")
            for b in range(BL)
        ]

        HH = HW // 2  # 2048, half of the hw axis
        for b in range(BL):
            val_tiles = {}
            for half in range(2):
                # ---- stage 1 for this half (2 groups of G=1024) ----
                for gi in range(2):
                    g = half * 2 + gi
                    key_sb = keyp.tile([128, TPG, D], bf16, tag="key")
                    nc.gpsimd.dma_start(
                        key_sb[:],
                        key_d[b, g * G:(g + 1) * G, :].rearrange(
                            "(t p) d -> p t d", p=128
                        ),
                    )
                    # prefetch val halves: 2 per group -> 8 per batch
                    for vi in range(2):
                        cc = gi * 2 + vi
                        vt = valp.tile(
                            [128, HH], bf16, tag="val",
                            name=f"val_{b}_{half}_{cc}",
                        )
                        nc.gpsimd.dma_start(
                            vt[:],
                            val_d[b, cc * 128:(cc + 1) * 128,
                                  half * HH:(half + 1) * HH],
                        )
                        val_tiles[(half, cc)] = vt
                    eT = eTp.tile([128, 4, G], bf16, tag="eT")
                    for k in range(4):
                        psT_t = psT.tile([128, G], bf16, tag="pst")
                        for t in range(TPG):
                            nc.tensor.transpose(
                                psT_t[:, t * 128:(t + 1) * 128],
                                key_sb[:, t, k * 128:(k + 1) * 128],
                                identity,
                            )
                        nc.scalar.activation(
                            eT[:, k, :],
                            psT_t[:],
                            AF.Tanh,
                            bias=qT[:, (k * BL + b):(k * BL + b) + 1],
                        )
                    for c in range(G // 512):
                        ps_s = psS.tile([BL, 512], f32, tag="pss")
                        for k in range(4):
                            nc.tensor.matmul(
                                ps_s[:],
                                wap[:, b, k, :],
                                eT[:, k, c * 512:(c + 1) * 512],
                                start=(k == 0),
                                stop=False,
                            )
                        off = g * G + c * 512
                        # accumulate log(mask) into row b (K=1 matmul)
                        nc.tensor.matmul(
                            ps_s[:],
                            cols4[:, b, :],
                            lmask1[:, b * HW + off:b * HW + off + 512],
                            start=False,
                            stop=True,
                        )
                        # exp straight off PSUM; rows != b are junk that
                        # the downstream selector matmuls zero out
                        ci = b * 8 + g * 2 + c
                        nc.scalar.activation(
                            u_bt[b][:, off:off + 512],
                            ps_s[:],
                            AF.Exp,
                            bias=ba4[:],
                            accum_out=den_all[:, ci:ci + 1],
                        )

                # ---- broadcast the (unnormalized) numerator ----
                sbc = sbcp.tile([128, HH], bf16, tag="sbc")
                for h2 in range(2):
                    ps_b = psB.tile([128, 1024], f32, tag="psb")
                    for j in range(2):
                        off = half * HH + h2 * 1024 + j * 512
                        nc.tensor.matmul(
                            ps_b[:, j * 512:(j + 1) * 512],
                            sel[:, b, :],
                            u_bt[b][:, off:off + 512],
                            start=True,
                            stop=True,
                        )
                    nc.scalar.activation(
                        sbc[:, h2 * 1024:(h2 + 1) * 1024], ps_b[:], AF.Copy
                    )

                # ---- stage 2 partial reductions for this half ----
                for cc in range(4):
                    val_sb = val_tiles[(half, cc)]
                    prod = scrp.tile([128, HH], bf16, tag="prod")
                    sum2 = scrp.tile([128, HH // 2], bf16, tag="sum2")
                    idx = b * 4 + cc
                    nc.vector.tensor_tensor(
                        prod[:], val_sb[:], sbc[:], OP.mult
                    )
                    nc.vector.tensor_tensor(
                        sum2[:], prod[:, 0:HH // 2], prod[:, HH // 2:HH],
                        OP.add,
                    )
                    nc.vector.tensor_reduce(
                        ctp[:, idx, half:half + 1],
                        sum2[:],
                        mybir.AxisListType.X,
                        OP.add,
                    )

            # ---- per-batch epilogue: denominator, recip, s output ----
            denb = up.tile([BL, 1], f32, tag="denb")
            nc.vector.tensor_reduce(
                denb[:], den_all[:, b * 8:(b + 1) * 8],
                mybir.AxisListType.X, OP.add,
            )
            den2 = up.tile([BL, 1], f32, tag="den2")
            nc.vector.tensor_scalar_add(den2[:], denb[:], 1e-10)
            recipb = up.tile([BL, 1], f32, tag="recipb")
            nc.vector.reciprocal(recipb[:], den2[:])
            # rdiag[:, b*4:(b+1)*4]: row r = recipb[r] * diag -> row b valid
            nc.vector.tensor_scalar_mul(
                rdiag[:, b * 4:(b + 1) * 4],
                diag4[:, b * 4:(b + 1) * 4],
                recipb[:],
            )
            un = up.tile([BL, HW], bf16, tag="un")
            nc.vector.tensor_scalar_mul(un[:], u_bt[b][:], recipb[:])
            nc.gpsimd.dma_start(s_d[b:b + 1, :], un[b:b + 1, :])

        # ---- tail: combine ct halves and normalize ----
        nc.vector.tensor_tensor(
            ct_sb[:], ctp[:, :, 0], ctp[:, :, 1], OP.add
        )
        ps_n = psB.tile([128, 1024], f32, tag="psb")
        nc.tensor.matmul(
            ps_n[:, 0:BL * 4], ones4[:], rdiag[:], start=True, stop=True
        )
        nc.vector.tensor_tensor(
            ct_sb[:], ct_sb[:], ps_n[:, 0:BL * 4], OP.mult
        )
        nc.sync.dma_start(
            ct_d.rearrange("b (c p) -> p b c", p=128),
            ct_sb.rearrange("p (b c) -> p b c", b=BL),
        )

    nc.compile()
    return nc


def _prep_in_maps(ctx_val, ctx_key, ctx_mask, ht_query, Wq, Wa, ba):
    key = np.ascontiguousarray(ctx_key.reshape(B, HW, D), dtype=np.float32)
    val = np.ascontiguousarray(ctx_val.reshape(B, C, HW), dtype=np.float32)
    mask = np.asarray(ctx_mask, dtype=np.float32).reshape(B, HW)
    with np.errstate(divide="ignore"):
        lmask = np.where(mask > 0, np.log(np.maximum(mask, 1e-38)), -1e30)
    lmask = np.ascontiguousarray(lmask, dtype=np.float32)
    ht = np.asarray(ht_query, dtype=np.float32)
    Wq = np.asarray(Wq, dtype=np.float32)
    Wa = np.asarray(Wa, dtype=np.float32)
    ba = np.asarray(ba, dtype=np.float32)

    # WqT4[n_local, nk, d] = Wq[d, nk*128 + n_local]
    WqT4 = np.ascontiguousarray(
        Wq.T.reshape(4, 128, D).transpose(1, 0, 2), dtype=np.float32
    )
    # wa_pad[d_local, b, k, col] = Wa[0, k*128+d_local] if col == b else 0
    waT = Wa[0].reshape(4, 128).T  # [d_local, k]
    wa_pad = np.zeros((128, BL, 4, BL), dtype=np.float32)
    for b in range(BL):
        wa_pad[:, b, :, b] = waT
    # sel[b, p, m] = 1 if p == b
    sel = np.zeros((BL, BL, 128), dtype=np.float32)
    for b in range(BL):
        sel[b, b, :] = 1.0
    ba4 = np.full((BL, 1), float(ba[0]), dtype=np.float32)
    # diag4[p, (b, cc)] = 1 if p == b (for the end-of-kernel ct normalize)
    diag4 = np.zeros((BL, BL * 4), dtype=np.float32)
    for b in range(BL):
        diag4[b, b * 4:(b + 1) * 4] = 1.0
    # cols4[0, b, j] = 1 if j == b (routes the logmask row into score row b)
    cols4 = np.zeros((1, BL, BL), dtype=np.float32)
    for b in range(BL):
        cols4[0, b, b] = 1.0
    # bmask[r, (b, gc)] = 1 if r == b (extracts valid chunk denominators)
    bmask = np.zeros((BL, BL * 8), dtype=np.float32)
    for b in range(BL):
        bmask[b, b * 8:(b + 1) * 8] = 1.0

    in_maps = []
    for core in range(NCORES):
        sl = slice(core * BL, (core + 1) * BL)
        ht_sh = ht[sl]  # [BL, N]
        # htT4[n_local, nk, b] = ht_sh[b, nk*128 + n_local]
        htT4 = np.ascontiguousarray(
            ht_sh.T.reshape(4, 128, BL).transpose(1, 0, 2), dtype=np.float32
        )
        in_maps.append(
            {
                "key": key[sl],
                "val": val[sl],
                "lmask": lmask[sl],
                "htT4": htT4,
                "WqT4": WqT4,
                "wa_pad": wa_pad,
                "sel": sel,
                "ba4": ba4,
                "diag4": diag4,
                "cols4": cols4,
                "bmask": bmask,
            }
        )
    return in_maps


def _install_profile_shim():
    """Provide antenv.axon_hooks + disable artifact upload so that
    run_bass_kernel_spmd(trace=True) can capture NTFF profiles in this
    container (the boot-time hook install is absent here)."""
    import types
    import ctypes
    import contextlib

    if "antenv.axon_hooks" not in sys.modules:
        mod = types.ModuleType("antenv.axon_hooks")
        holder = {"h": None}
        mod.set_axon_ntff_profile_hook = lambda h: holder.update(h=h)
        mod.get_axon_ntff_profile_hook = lambda: holder["h"]
        sys.modules["antenv.axon_hooks"] = mod
        import antenv

        antenv.axon_hooks = mod

    from antenv.axon_hooks import (
        get_axon_ntff_profile_hook,
        set_axon_ntff_profile_hook,
    )

    if get_axon_ntff_profile_hook() is None:
        lib = ctypes.CDLL("/opt/axon/libaxon_pjrt.so")
        if hasattr(lib, "axon_start_nrt_profile"):
            lib.axon_start_nrt_profile.argtypes = [
                ctypes.POINTER(ctypes.c_int64),
                ctypes.c_size_t,
            ]
            lib.axon_start_nrt_profile.restype = ctypes.c_int64
            lib.axon_stop_nrt_profile.argtypes = [ctypes.c_char_p]
            lib.axon_stop_nrt_profile.restype = ctypes.c_int64

            @contextlib.contextmanager
            def _hook(output_dir, device_ids):
                import jax

                jax.devices()
                if device_ids:
                    ids = (ctypes.c_int64 * len(device_ids))(*device_ids)
                    rc = lib.axon_start_nrt_profile(ids, len(device_ids))
                else:
                    rc = lib.axon_start_nrt_profile(None, 0)
                if rc != 0:
                    raise RuntimeError(f"axon_start_nrt_profile rc={rc}")
                try:
                    yield
                finally:
                    n = lib.axon_stop_nrt_profile(str(output_dir).encode())
                    print(f"profile: {n} file(s) written to {output_dir}")

            set_axon_ntff_profile_hook(_hook)

    from concourse import bass_utils as bu

    bu.upload_artifacts = lambda tmpdir: f"local:{tmpdir}"


def kernel(ctx_val, ctx_key, ctx_mask, ht_query, Wq, Wa, ba, _trace=False):
    from concourse.bass_utils import run_bass_kernel_spmd

    if _trace:
        _install_profile_shim()

    if "nc" not in _CACHE:
        _CACHE["nc"] = _build_nc()
    nc = _CACHE["nc"]

    in_maps = _prep_in_maps(ctx_val, ctx_key, ctx_mask, ht_query, Wq, Wa, ba)
    res = run_bass_kernel_spmd(
        nc, in_maps, core_ids=list(range(NCORES)), trace=_trace
    )
    if _trace:
        print(f"HW exec time: {res.exec_time_ns} ns")
        _CACHE["exec_time_ns"] = res.exec_time_ns
        _CACHE["results_obj"] = res

    ct = np.concatenate([r["out_ct"] for r in res.results], axis=0)
    s = np.concatenate([r["out_s"] for r in res.results], axis=0)
    return ct.astype(np.float32), s.reshape(B, H, W).astype(np.float32)


# revision 32
# speedup vs baseline: 1.0906x; 1.0171x over previous
"""Trainium2 Bass kernel for nn_Attention_16355235463288.

Additive attention:
    q  = ht_query @ Wq.T                      [B, D]
    e  = tanh(ctx_key + q[:, None, None, :])  [B, H, W, D]
    s  = einsum('bhwd,d->bhw', e, Wa[0]) + ba [B, H, W]
    s  = exp(s - max(s)) * mask ; s /= (sum_hw(s) + 1e-10)
    ct = einsum('bchw,bhw->bc', ctx_val, s)   [B, C]
    returns (ct, s)

Sharding: pure data parallel, B=32 over 8 cores (BL=4 per core). Params
replicated. No collectives. The global max-subtract in the reference is
dropped: softmax ratios are invariant to the subtracted constant except
through the +1e-10 term, where the relative effect is ~1e-11; |s| is
bounded by sum|Wa| ~ 20 so exp() cannot overflow in fp32. The mask is
folded in as a host-precomputed log(mask) initializer of the score
accumulator, so exp(s + logmask) = exp(s) * mask with no mask multiply.

Per-core dataflow (each stage streams ~33.5 MB from HBM), per batch b so
stage 2 of batch b overlaps stage 1 of batch b+1:
  stage 0: qT[d, b] via PE matmul from host-pretransposed WqT/htT (fp32).
  stage 1 (ctx_key):
    SWDGE cast-loads key tiles [128(hw), 512(d)] f32->bf16
    -> PE transpose 128x128 bf16 blocks into PSUM [128(d), hw]
    -> ACT tanh(psum + qT_bias) -> bf16 SBUF (fused q-add)
    -> PE matmul with zero-padded-column Wa weights -> scores [4, 512]
    -> DVE in-place add into s_sb[4, HW] (rows != b get += 0).
  softmax(b): ACT exp(s_sb + ba) -> u bf16 with accum_out giving the
    denominator for free; DVE reciprocal; tiny PE matmul broadcasts
    recip[b] over 128 partitions.
  stage 2 (ctx_val):
    PE selector matmul broadcasts u[b] over 128 partitions
    -> ACT Copy with scale=recip128 (fused normalize) -> ubc bf16
    -> SWDGE cast-load val tiles [128(c), 4096(hw)] f32->bf16
    -> DVE mul + reduce over hw -> ct column.
  tail: one fp32 exp + normalize for the s output.

tensor_tensor_reduce is avoided: it faults the NRT runtime on this
fleet (bisected 2026-08-03); plain tensor_tensor + tensor_reduce works.
"""

import sys
import numpy as np

for _p in ("/opt/trn_rl_repo", "/opt/pypackages"):
    if _p not in sys.path:
        sys.path.append(_p)

B, H, W, D, N, C = 32, 64, 64, 512, 512, 512
NCORES = 8
BL = B // NCORES          # 4 batches per core
HW = H * W                # 4096
G = 1024                  # hw positions per stage-1 group
NG = HW // G              # 4 groups
TPG = G // 128            # 8 hw-tiles per group

_CACHE = {}


def _build_nc():
    import concourse.bass as bass
    import concourse.mybir as mybir
    import concourse.tile as tile
    from concourse import bacc
    from concourse.masks import make_identity
    from contextlib import ExitStack

    f32 = mybir.dt.float32
    bf16 = mybir.dt.bfloat16
    AF = mybir.ActivationFunctionType
    OP = mybir.AluOpType

    nc = bacc.Bacc(None, target_bir_lowering=False, debug=False)

    key_d = nc.declare_dram_parameter("key", [BL, HW, D], f32, isOutput=False)
    val_d = nc.declare_dram_parameter("val", [BL, C, HW], f32, isOutput=False)
    lmask_d = nc.declare_dram_parameter("lmask", [BL, HW], f32, isOutput=False)
    htT_d = nc.declare_dram_parameter("htT4", [128, 4, BL], f32, isOutput=False)
    wqT_d = nc.declare_dram_parameter("WqT4", [128, 4, D], f32, isOutput=False)
    wap_d = nc.declare_dram_parameter("wa_pad", [128, BL, 4, BL], f32, isOutput=False)
    sel_d = nc.declare_dram_parameter("sel", [BL, BL, 128], f32, isOutput=False)
    ba_d = nc.declare_dram_parameter("ba4", [BL, 1], f32, isOutput=False)
    dg_d = nc.declare_dram_parameter("diag4", [BL, BL * 4], f32, isOutput=False)
    c4_d = nc.declare_dram_parameter("cols4", [1, BL, BL], f32, isOutput=False)
    bm_d = nc.declare_dram_parameter("bmask", [BL, BL * 8], f32, isOutput=False)
    ct_d = nc.declare_dram_parameter("out_ct", [BL, C], f32, isOutput=True)
    s_d = nc.declare_dram_parameter("out_s", [BL, HW], f32, isOutput=True)

    with tile.TileContext(nc) as tc, ExitStack() as ctx:
        const = ctx.enter_context(tc.tile_pool(name="const", bufs=1))
        keyp = ctx.enter_context(tc.tile_pool(name="keyp", bufs=4))
        eTp = ctx.enter_context(tc.tile_pool(name="eTp", bufs=2))
        valp = ctx.enter_context(tc.tile_pool(name="valp", bufs=10))
        sbcp = ctx.enter_context(tc.tile_pool(name="sbcp", bufs=2))
        scrp = ctx.enter_context(tc.tile_pool(name="scrp", bufs=2))
        smallp = ctx.enter_context(tc.tile_pool(name="smallp", bufs=1))
        up = ctx.enter_context(tc.tile_pool(name="up", bufs=2))
        psT = ctx.enter_context(tc.tile_pool(name="psT", bufs=3, space="PSUM"))
        psS = ctx.enter_context(tc.tile_pool(name="psS", bufs=2, space="PSUM"))
        psB = ctx.enter_context(tc.tile_pool(name="psB", bufs=2, space="PSUM"))

        # start the bulk stream immediately; consts fill DMA gaps later
        pre_key = keyp.tile([128, TPG, D], bf16, tag="key", name="pre_key")
        nc.gpsimd.dma_start(
            pre_key[:],
            key_d[0, 0:G, :].rearrange("(t p) d -> p t d", p=128),
        )
        pre_val = {}
        for vi in range(2):
            pv = valp.tile(
                [128, HW // 2], bf16, tag="val", name=f"pre_val_{vi}"
            )
            nc.gpsimd.dma_start(
                pv[:], val_d[0, vi * 128:(vi + 1) * 128, 0:HW // 2]
            )
            pre_val[vi] = pv

        identity = const.tile([128, 128], bf16)
        make_identity(nc, identity)
        htT = const.tile([128, 4, BL], f32)
        nc.sync.dma_start(htT[:], htT_d[:])
        wqT = const.tile([128, 4, D], f32)
        nc.sync.dma_start(wqT[:], wqT_d[:])
        wap = const.tile([128, BL, 4, BL], bf16)
        nc.gpsimd.dma_start(wap[:], wap_d[:])
        sel = const.tile([BL, BL, 128], bf16)
        nc.gpsimd.dma_start(sel[:], sel_d[:])
        ba4 = const.tile([BL, 1], f32)
        nc.sync.dma_start(ba4[:], ba_d[:])
        diag4 = const.tile([BL, BL * 4], f32)
        nc.sync.dma_start(diag4[:], dg_d[:])
        ones4 = const.tile([BL, 128], f32)
        nc.vector.memset(ones4[:], 1.0)
        cols4 = const.tile([1, BL, BL], bf16)
        nc.gpsimd.dma_start(cols4[:], c4_d[:])
        bmask = const.tile([BL, BL * 8], f32)
        nc.sync.dma_start(bmask[:], bm_d[:])
        lmask1 = const.tile([1, BL * HW], bf16)
        nc.gpsimd.dma_start(lmask1[:], lmask_d[:].rearrange("b h -> (b h)"))
        ct_sb = const.tile([128, BL * 4], f32)
        ctp = const.tile([128, BL * 4, 2], f32)

        # stage 0: qT[d_local, (k, b)] = sum_n Wq[d, n] * ht[b, n]
        ps_q = psS.tile([128, 4 * BL], f32, tag="pss")
        for k in range(4):
            for nk in range(4):
                nc.tensor.matmul(
                    ps_q[:, k * BL:(k + 1) * BL],
                    wqT[:, nk, k * 128:(k + 1) * 128],
                    htT[:, nk, :],
                    start=(nk == 0),
                    stop=(nk == 3),
                )
        qT = const.tile([128, 4 * BL], f32)
        nc.vector.tensor_copy(qT[:], ps_q[:])

        denom = smallp.tile([BL, 1], f32, tag="denom")
        recip = smallp.tile([BL, 1], f32, tag="recip")
        den_all = smallp.tile([BL, BL * 8], f32, tag="den_all")
        rdiag = smallp.tile([BL, BL * 4], f32, tag="rdiag")
        u_bt = [
            smallp.tile([BL, HW], bf16, tag=f"u{b}", name=f"u_{b}")
            for b in range(BL)
        ]

        HH = HW // 2  # 2048, half of the hw axis
        for b in range(BL):
            val_tiles = {}
            for half in range(2):
                # ---- stage 1 for this half (2 groups of G=1024) ----
                for gi in range(2):
                    g = half * 2 + gi
                    if b == 0 and g == 0:
                        key_sb = pre_key
                    else:
                        key_sb = keyp.tile([128, TPG, D], bf16, tag="key")
                        nc.gpsimd.dma_start(
                            key_sb[:],
                            key_d[b, g * G:(g + 1) * G, :].rearrange(
                                "(t p) d -> p t d", p=128
                            ),
                        )
                    # prefetch val halves: 2 per group -> 8 per batch
                    for vi in range(2):
                        cc = gi * 2 + vi
                        if b == 0 and g == 0:
                            val_tiles[(half, cc)] = pre_val[vi]
                            continue
                        vt = valp.tile(
                            [128, HH], bf16, tag="val",
                            name=f"val_{b}_{half}_{cc}",
                        )
                        nc.gpsimd.dma_start(
                            vt[:],
                            val_d[b, cc * 128:(cc + 1) * 128,
                                  half * HH:(half + 1) * HH],
                        )
                        val_tiles[(half, cc)] = vt
                    eT = eTp.tile([128, 4, G], bf16, tag="eT")
                    for k in range(4):
                        psT_t = psT.tile([128, G], bf16, tag="pst")
                        for t in range(TPG):
                            nc.tensor.transpose(
                                psT_t[:, t * 128:(t + 1) * 128],
                                key_sb[:, t, k * 128:(k + 1) * 128],
                                identity,
                            )
                        nc.scalar.activation(
                            eT[:, k, :],
                            psT_t[:],
                            AF.Tanh,
                            bias=qT[:, (k * BL + b):(k * BL + b) + 1],
                        )
                    for c in range(G // 512):
                        ps_s = psS.tile([BL, 512], f32, tag="pss")
                        for k in range(4):
                            nc.tensor.matmul(
                                ps_s[:],
                                wap[:, b, k, :],
                                eT[:, k, c * 512:(c + 1) * 512],
                                start=(k == 0),
                                stop=False,
                            )
                        off = g * G + c * 512
                        # accumulate log(mask) into row b (K=1 matmul)
                        nc.tensor.matmul(
                            ps_s[:],
                            cols4[:, b, :],
                            lmask1[:, b * HW + off:b * HW + off + 512],
                            start=False,
                            stop=True,
                        )
                        # exp straight off PSUM; rows != b are junk that
                        # the downstream selector matmuls zero out
                        ci = b * 8 + g * 2 + c
                        nc.scalar.activation(
                            u_bt[b][:, off:off + 512],
                            ps_s[:],
                            AF.Exp,
                            bias=ba4[:],
                            accum_out=den_all[:, ci:ci + 1],
                        )

                # ---- broadcast the (unnormalized) numerator ----
                sbc = sbcp.tile([128, HH], bf16, tag="sbc")
                for h2 in range(4):
                    ps_b = psB.tile([128, 512], f32, tag="psb")
                    off = half * HH + h2 * 512
                    nc.tensor.matmul(
                        ps_b[:],
                        sel[:, b, :],
                        u_bt[b][:, off:off + 512],
                        start=True,
                        stop=True,
                    )
                    nc.scalar.activation(
                        sbc[:, h2 * 512:(h2 + 1) * 512], ps_b[:], AF.Copy
                    )

                # ---- stage 2 partial reductions for this half ----
                for cc in range(4):
                    val_sb = val_tiles[(half, cc)]
                    prod = scrp.tile([128, HH], bf16, tag="prod")
                    sum2 = scrp.tile([128, HH // 2], bf16, tag="sum2")
                    idx = b * 4 + cc
                    nc.vector.tensor_tensor(
                        prod[:], val_sb[:], sbc[:], OP.mult
                    )
                    nc.vector.tensor_tensor(
                        sum2[:], prod[:, 0:HH // 2], prod[:, HH // 2:HH],
                        OP.add,
                    )
                    nc.vector.tensor_reduce(
                        ctp[:, idx, half:half + 1],
                        sum2[:],
                        mybir.AxisListType.X,
                        OP.add,
                    )

            # ---- per-batch epilogue: denominator, recip, s output ----
            denb = up.tile([BL, 1], f32, tag="denb")
            nc.vector.tensor_reduce(
                denb[:], den_all[:, b * 8:(b + 1) * 8],
                mybir.AxisListType.X, OP.add,
            )
            den2 = up.tile([BL, 1], f32, tag="den2")
            nc.vector.tensor_scalar_add(den2[:], denb[:], 1e-10)
            recipb = up.tile([BL, 1], f32, tag="recipb")
            nc.vector.reciprocal(recipb[:], den2[:])
            # rdiag[:, b*4:(b+1)*4]: row r = recipb[r] * diag -> row b valid
            nc.vector.tensor_scalar_mul(
                rdiag[:, b * 4:(b + 1) * 4],
                diag4[:, b * 4:(b + 1) * 4],
                recipb[:],
            )
            un = up.tile([BL, HW], bf16, tag="un")
            nc.vector.tensor_scalar_mul(un[:], u_bt[b][:], recipb[:])
            nc.gpsimd.dma_start(s_d[b:b + 1, :], un[b:b + 1, :])

        # ---- tail: combine ct halves and normalize ----
        nc.vector.tensor_tensor(
            ct_sb[:], ctp[:, :, 0], ctp[:, :, 1], OP.add
        )
        ps_n = psB.tile([128, 512], f32, tag="psb")
        nc.tensor.matmul(
            ps_n[:, 0:BL * 4], ones4[:], rdiag[:], start=True, stop=True
        )
        nc.vector.tensor_tensor(
            ct_sb[:], ct_sb[:], ps_n[:, 0:BL * 4], OP.mult
        )
        nc.sync.dma_start(
            ct_d.rearrange("b (c p) -> p b c", p=128),
            ct_sb.rearrange("p (b c) -> p b c", b=BL),
        )

    nc.compile()
    return nc


def _prep_in_maps(ctx_val, ctx_key, ctx_mask, ht_query, Wq, Wa, ba):
    key = np.ascontiguousarray(ctx_key.reshape(B, HW, D), dtype=np.float32)
    val = np.ascontiguousarray(ctx_val.reshape(B, C, HW), dtype=np.float32)
    mask = np.asarray(ctx_mask, dtype=np.float32).reshape(B, HW)
    with np.errstate(divide="ignore"):
        lmask = np.where(mask > 0, np.log(np.maximum(mask, 1e-38)), -1e30)
    lmask = np.ascontiguousarray(lmask, dtype=np.float32)
    ht = np.asarray(ht_query, dtype=np.float32)
    Wq = np.asarray(Wq, dtype=np.float32)
    Wa = np.asarray(Wa, dtype=np.float32)
    ba = np.asarray(ba, dtype=np.float32)

    # WqT4[n_local, nk, d] = Wq[d, nk*128 + n_local]
    WqT4 = np.ascontiguousarray(
        Wq.T.reshape(4, 128, D).transpose(1, 0, 2), dtype=np.float32
    )
    # wa_pad[d_local, b, k, col] = Wa[0, k*128+d_local] if col == b else 0
    waT = Wa[0].reshape(4, 128).T  # [d_local, k]
    wa_pad = np.zeros((128, BL, 4, BL), dtype=np.float32)
    for b in range(BL):
        wa_pad[:, b, :, b] = waT
    # sel[b, p, m] = 1 if p == b
    sel = np.zeros((BL, BL, 128), dtype=np.float32)
    for b in range(BL):
        sel[b, b, :] = 1.0
    ba4 = np.full((BL, 1), float(ba[0]), dtype=np.float32)
    # diag4[p, (b, cc)] = 1 if p == b (for the end-of-kernel ct normalize)
    diag4 = np.zeros((BL, BL * 4), dtype=np.float32)
    for b in range(BL):
        diag4[b, b * 4:(b + 1) * 4] = 1.0
    # cols4[0, b, j] = 1 if j == b (routes the logmask row into score row b)
    cols4 = np.zeros((1, BL, BL), dtype=np.float32)
    for b in range(BL):
        cols4[0, b, b] = 1.0
    # bmask[r, (b, gc)] = 1 if r == b (extracts valid chunk denominators)
    bmask = np.zeros((BL, BL * 8), dtype=np.float32)
    for b in range(BL):
        bmask[b, b * 8:(b + 1) * 8] = 1.0

    in_maps = []
    for core in range(NCORES):
        sl = slice(core * BL, (core + 1) * BL)
        ht_sh = ht[sl]  # [BL, N]
        # htT4[n_local, nk, b] = ht_sh[b, nk*128 + n_local]
        htT4 = np.ascontiguousarray(
            ht_sh.T.reshape(4, 128, BL).transpose(1, 0, 2), dtype=np.float32
        )
        in_maps.append(
            {
                "key": key[sl],
                "val": val[sl],
                "lmask": lmask[sl],
                "htT4": htT4,
                "WqT4": WqT4,
                "wa_pad": wa_pad,
                "sel": sel,
                "ba4": ba4,
                "diag4": diag4,
                "cols4": cols4,
                "bmask": bmask,
            }
        )
    return in_maps


def _install_profile_shim():
    """Provide antenv.axon_hooks + disable artifact upload so that
    run_bass_kernel_spmd(trace=True) can capture NTFF profiles in this
    container (the boot-time hook install is absent here)."""
    import types
    import ctypes
    import contextlib

    if "antenv.axon_hooks" not in sys.modules:
        mod = types.ModuleType("antenv.axon_hooks")
        holder = {"h": None}
        mod.set_axon_ntff_profile_hook = lambda h: holder.update(h=h)
        mod.get_axon_ntff_profile_hook = lambda: holder["h"]
        sys.modules["antenv.axon_hooks"] = mod
        import antenv

        antenv.axon_hooks = mod

    from antenv.axon_hooks import (
        get_axon_ntff_profile_hook,
        set_axon_ntff_profile_hook,
    )

    if get_axon_ntff_profile_hook() is None:
        lib = ctypes.CDLL("/opt/axon/libaxon_pjrt.so")
        if hasattr(lib, "axon_start_nrt_profile"):
            lib.axon_start_nrt_profile.argtypes = [
                ctypes.POINTER(ctypes.c_int64),
                ctypes.c_size_t,
            ]
            lib.axon_start_nrt_profile.restype = ctypes.c_int64
            lib.axon_stop_nrt_profile.argtypes = [ctypes.c_char_p]
            lib.axon_stop_nrt_profile.restype = ctypes.c_int64

            @contextlib.contextmanager
            def _hook(output_dir, device_ids):
                import jax

                jax.devices()
                if device_ids:
                    ids = (ctypes.c_int64 * len(device_ids))(*device_ids)
                    rc = lib.axon_start_nrt_profile(ids, len(device_ids))
                else:
                    rc = lib.axon_start_nrt_profile(None, 0)
                if rc != 0:
                    raise RuntimeError(f"axon_start_nrt_profile rc={rc}")
                try:
                    yield
                finally:
                    n = lib.axon_stop_nrt_profile(str(output_dir).encode())
                    print(f"profile: {n} file(s) written to {output_dir}")

            set_axon_ntff_profile_hook(_hook)

    from concourse import bass_utils as bu

    bu.upload_artifacts = lambda tmpdir: f"local:{tmpdir}"


def kernel(ctx_val, ctx_key, ctx_mask, ht_query, Wq, Wa, ba, _trace=False):
    from concourse.bass_utils import run_bass_kernel_spmd

    if _trace:
        _install_profile_shim()

    if "nc" not in _CACHE:
        _CACHE["nc"] = _build_nc()
    nc = _CACHE["nc"]

    in_maps = _prep_in_maps(ctx_val, ctx_key, ctx_mask, ht_query, Wq, Wa, ba)
    res = run_bass_kernel_spmd(
        nc, in_maps, core_ids=list(range(NCORES)), trace=_trace
    )
    if _trace:
        print(f"HW exec time: {res.exec_time_ns} ns")
        _CACHE["exec_time_ns"] = res.exec_time_ns
        _CACHE["results_obj"] = res

    ct = np.concatenate([r["out_ct"] for r in res.results], axis=0)
    s = np.concatenate([r["out_s"] for r in res.results], axis=0)
    return ct.astype(np.float32), s.reshape(B, H, W).astype(np.float32)
